# revision 1
# baseline (speedup 1.0000x reference)
"""Trainium2 kernel for the Applied-Hamiltonian derivative problem.

Math (see reference):
    H = H0 + H1(t),  H1 = sum_i kron(I, s_i, I) with s_i complex 2x2 per qubit site
    dUr = (H0 + Hr) @ Ui + Hi @ Ur
    dUi = Hi @ Ui - (H0 + Hr) @ Ur

Structure exploited:
  * Hr and Hi are sparse (<= 12 nonzeros/row: a diagonal plus one off-diagonal
    per site at stride 2^k).  Hr is folded into G = H0 + Hr on the host
    (cheap scatter-add), leaving exactly 2 dense 2048^3 GEMMs on the device.
  * Hi's action decomposes per 128-row tile T as
        (Hi @ X)[T] = L_T @ X[T] + sum_{j<4} c_j(T) * X[T ^ e_j]
    where L_T is a 128x128 matrix (low sites + diagonal) and the 4 high
    sites are scalar couplings between row tiles.  L_T rides the dense PSUM
    chain as one extra TensorE matmul (17 instead of 16 per 128x512 output
    tile); the high-site part W = sum_j c_j * X[T^e_j] is combined on the
    otherwise-idle VectorE and added during the PSUM->SBUF epilogue, off the
    TensorE critical path.
  * Shipping Urneg = -Ur lets both output planes come straight out of PSUM
    with no epilogue negation.

Sharding: 2 row-groups x 4 col-groups over 8 cores.  Each core computes
out[p*1024:(p+1)*1024, q*512:(q+1)*512] for both planes.  To keep the SPMD
graph identical across cores, the K row-tiles of gt/ui/urn are XOR-permuted
by 8*p on the host so tile-partner indices are core-independent.

Compute dtype bf16 (inputs pre-cast on host), accumulation fp32 in PSUM.
"""

import numpy as np
import ml_dtypes

import concourse.bass as bass
import concourse.mybir as mybir
import concourse.tile as tile
from concourse.bass_utils import run_bass_kernel_spmd

T_TOTAL = 10.0
N_SITES = 11
DIM = 2048
P = 128
NT = DIM // P          # 16 row/k tiles of the full problem
PR, PC = 2, 4          # row groups x col groups = 8 cores
ROWS = DIM // PR       # 1024 output rows per core
COLS = DIM // PC       # 512 output cols per core
LT = ROWS // P         # 8 output row-tiles per core
BF16 = mybir.dt.bfloat16
F32 = mybir.dt.float32
BF = ml_dtypes.bfloat16

_NC_CACHE = None
_RUN_KWARGS = {}    # test harness can inject trace=True etc.
_LAST_RESULT = None  # BassKernelResults of the most recent run


def _build_graph():
    nc = bass.Bass()
    # gt/ui/urn are shipped already in SBUF layout [128, NT, *] so every DMA
    # is one contiguous descriptor per partition (fast HWDGE issue).
    gu_ext = nc.declare_dram_parameter(
        "gu", [P, NT, ROWS + 2 * COLS], BF16, isOutput=False)
    lci_ext = nc.declare_dram_parameter("lci", [P, 2, LT, P], BF16, isOutput=False)
    # per-(sign, tile, site) high-site coefficients for the DVE combinations
    ch_ext = nc.declare_dram_parameter("ch", [P, 2, LT, 4], F32, isOutput=False)
    out_ext = nc.declare_dram_parameter("out", [2, ROWS, COLS], F32, isOutput=True)

    out_pv = out_ext[:].rearrange("s (tp h p) n -> s tp p h n", p=P, h=2)
    out_tv = out_ext[:].rearrange("s (tl p) n -> s tl p n", p=P)

    with tile.TileContext(nc) as tc:
        with (
            tc.tile_pool(name="big", bufs=1) as big,
            tc.tile_pool(name="outp", bufs=8) as outp,
            tc.tile_pool(name="wp", bufs=16) as wpool,
            tc.tile_pool(name="tp", bufs=4) as tpool,
            tc.tile_pool(name="psum", bufs=8, space="PSUM") as psump,
        ):
            gu_sb = big.tile([P, NT, ROWS + 2 * COLS], BF16, tag="gu")
            # [gtA | ui | urn | gtB]: wave A's weights travel with ui/urn so
            # the ramp's critical DMA bytes per k-tile shrink by 25%; the gtB
            # halves ship afterwards (wave B starts much later).
            ui_sb = gu_sb[:, :, 512:1024]
            urn_sb = gu_sb[:, :, 1024:1536]

            def gt_lhsT(kt, tl):
                off = tl * P if tl < 4 else 1536 + (tl - 4) * P
                return gu_sb[:, kt, off:off + P]
            lci_sb = big.tile([P, 2, LT, P], BF16, tag="lci")
            ch_sb = big.tile([P, 2, LT, 4], F32, tag="ch")

            # progressive granularity: tiny first chunks let the PE start
            # ~7us earlier; big tail chunks keep DMA issue overhead low.
            # gtA+ui+urn (wave A's working set) first, gtB halves afterwards.
            for gi, (lo, hi) in enumerate(
                    ((0, 1), (1, 2), (2, 4), (4, 6), (6, 8), (8, 10),
                     (10, 12), (12, 14), (14, 16))):
                sl = slice(lo, hi)
                nc.sync.dma_start(gu_sb[:, sl, 0:1536], gu_ext[:, sl, 0:1536])
                if gi == 0:
                    nc.sync.dma_start(lci_sb[:], lci_ext[:])
                    nc.sync.dma_start(ch_sb[:], ch_ext[:])
            for lo, hi in ((0, 8), (8, 16)):
                sl = slice(lo, hi)
                nc.sync.dma_start(gu_sb[:, sl, 1536:2048],
                                  gu_ext[:, sl, 1536:2048])

            # HAM warm-up: the PE clock-gate needs ~3.4us of sustained matmul
            # activity to reach 2.4 GHz.  The PE is idle from the end of the
            # preamble until the first k-tile lands (~4us), so burn that window
            # on dummy matmuls over memset scratch; real matmuls then issue at
            # full rate from the first k-tile.
            warm_lhs = tpool.tile([P, P], BF16, tag="wl", name="warm_lhs")
            warm_rhs = tpool.tile([P, COLS], BF16, tag="wr", name="warm_rhs")
            nc.gpsimd.memset(warm_lhs[:], 0.0)
            nc.gpsimd.memset(warm_rhs[:], 0.0)
            warm_ps = psump.tile([P, COLS], F32, tag="ps", name="warm_ps")
            for wi in range(10):
                nc.tensor.matmul(warm_ps[:], warm_lhs[:], warm_rhs[:],
                                 start=(wi == 0), stop=(wi == 9))

            # High-site combinations on DVE for every chain, ordered so ops
            # touching late-arriving tiles come last.  W is only needed at
            # epilogue time (og += W after the PSUM-releasing copy), so the
            # DVE has tens of microseconds of slack.
            wt = {}
            for tl in range(LT):
                for s in (0, 1):
                    src = urn_sb if s == 0 else ui_sb
                    t0 = tpool.tile([P, COLS], BF16, tag="t0", name=f"t0_{tl}_{s}")
                    t1 = tpool.tile([P, COLS], BF16, tag="t1", name=f"t1_{tl}_{s}")
                    nc.vector.tensor_scalar_mul(
                        t0[:], src[:, tl ^ 4], ch_sb[:, s, tl, 1:2])
                    nc.vector.tensor_scalar_mul(
                        t1[:], src[:, tl ^ 2], ch_sb[:, s, tl, 2:3])
                    nc.vector.tensor_add(t0[:], t0[:], t1[:])
                    nc.vector.tensor_scalar_mul(
                        t1[:], src[:, tl ^ 1], ch_sb[:, s, tl, 3:4])
                    nc.vector.tensor_add(t0[:], t0[:], t1[:])
                    nc.vector.tensor_scalar_mul(
                        t1[:], src[:, tl ^ 8], ch_sb[:, s, tl, 0:1])
                    w = wpool.tile([P, COLS], BF16, tag="w", name=f"w_{tl}_{s}")
                    nc.vector.tensor_add(w[:], t0[:], t1[:])
                    wt[tl, s] = w

            # Two waves of 8 PSUM chains (4 row-tiles x 2 planes), k-major so
            # the PE can consume k-tiles as the DMAs land.
            for wave in (range(0, 4), range(4, 8)):
                ps = {}
                for tl in wave:
                    for s in (0, 1):
                        ps[tl, s] = psump.tile([P, COLS], F32, tag="ps", name=f"ps_{tl}_{s}")
                for kt in range(NT):
                    for tl in wave:
                        lhsT = gt_lhsT(kt, tl)
                        nc.tensor.matmul(
                            ps[tl, 0][:], lhsT, ui_sb[:, kt],
                            start=(kt == 0), stop=False,
                        )
                        nc.tensor.matmul(
                            ps[tl, 1][:], lhsT, urn_sb[:, kt],
                            start=(kt == 0), stop=False,
                        )
                for tl in wave:
                    for s in (0, 1):
                        other = urn_sb if s == 0 else ui_sb
                        nc.tensor.matmul(
                            ps[tl, s][:], lci_sb[:, s, tl], other[:, tl],
                            start=False, stop=True,
                        )
                # per-(plane, row-tile) output: each 0.25 MiB DMA leaves only
                # a short tail after the last PSUM chain completes.
                if wave[0] == 0:
                    for s in (0, 1):
                        for tp in (0, 1):
                            og = outp.tile([P, 2, COLS], F32, tag="og",
                                           name=f"og_{tp}_{s}")
                            for h, tl in enumerate((2 * tp, 2 * tp + 1)):
                                # ACT copy releases the PSUM bank quickly so
                                # the next wave's matmuls can start; W lands
                                # later, off the critical path.
                                nc.scalar.copy(og[:, h], ps[tl, s][:])
                                nc.vector.tensor_add(og[:, h], og[:, h],
                                                     wt[tl, s][:])
                            nc.sync.dma_start(out_pv[s, tp], og[:])
                else:
                    # final wave: per-tile outputs so the last DMA after the
                    # last PSUM chain is only 0.25 MiB
                    for tl in wave:
                        for s in (0, 1):
                            og1 = outp.tile([P, COLS], F32, tag="og1",
                                            name=f"og1_{tl}_{s}")
                            nc.vector.tensor_add(og1[:], ps[tl, s][:],
                                                 wt[tl, s][:])
                            nc.sync.dma_start(out_tv[s, tl], og1[:])
    return nc


def _split_sync_waits(nc, cap=1):
    """Walrus's per-instruction sync-wait slots are limited (DMA DIRECT2D
    rejects 2, the final drain's 14 are far over).  Engines execute their
    stream serially, so hoisting excess waits into preceding NoOps on the
    same engine is semantically identical."""
    for fn in nc.m.functions:
        for bb in fn.blocks:
            new_insts = []
            for inst in bb.instructions:
                si = getattr(inst, "sync_info", None)
                waits = list(si.on_wait) if si is not None and si.on_wait else []
                if len(waits) > cap:
                    extra, keep = waits[:-cap], waits[-cap:]
                    for i in range(0, len(extra), cap):
                        new_insts.append(mybir.InstNoOp(
                            name=f"{inst.name}-wsplit{i}",
                            engine=inst.engine,
                            bass_nofuse=True,
                            sync_info=mybir.SyncInfo(
                                on_wait=extra[i:i + cap], on_update=[]),
                        ))
                    si.on_wait = keep
                new_insts.append(inst)
            bb.instructions[:] = new_insts


def _get_nc():
    global _NC_CACHE
    if _NC_CACHE is None:
        nc = _build_graph()
        _split_sync_waits(nc)
        _NC_CACHE = nc
    return _NC_CACHE


def _site_ops(A, gates_re, gates_im, t):
    M, NG = A.shape
    n_gates = gates_re.shape[0]
    nsites = NG // n_gates
    a = 0.5 * (T_TOTAL / M)
    tm = np.arange(M, dtype=np.float64) * (T_TOTAL / M)
    env = np.exp(-np.square(float(t) - tm) / (a * a))
    coef = (env @ A.astype(np.float64)).reshape(n_gates, nsites)
    site_re = np.einsum("gn,gab->nab", coef, gates_re.astype(np.float64))
    site_im = np.einsum("gn,gab->nab", coef, gates_im.astype(np.float64))
    return site_re, site_im


def kernel(A, gates_re, gates_im, H0, U, t):
    A = np.asarray(A)
    gates_re = np.asarray(gates_re)
    gates_im = np.asarray(gates_im)
    H0 = np.asarray(H0)
    U = np.asarray(U)
    t = float(np.asarray(t))

    site_re, site_im = _site_ops(A, gates_re, gates_im, t)
    nsites = N_SITES
    strides = [2 ** (nsites - 1 - i) for i in range(nsites)]
    r = np.arange(DIM)
    bits = [((r >> (nsites - 1 - i)) & 1) for i in range(nsites)]

    # G = H0 + Hr via scatter-add (Hr has <= 12 nonzeros per row)
    G = H0.astype(np.float32).copy()
    diag = np.zeros(DIM)
    for i in range(nsites):
        diag += site_re[i][bits[i], bits[i]]
    G[r, r] += diag.astype(np.float32)
    for i in range(nsites):
        G[r, r ^ strides[i]] += site_re[i][bits[i], 1 - bits[i]].astype(np.float32)

    # Per-tile low-site operators and high-site couplings of Hi
    p = np.arange(P)
    L = np.zeros((NT, P, P))
    chigh = np.zeros((NT, 4))
    dlow = np.zeros(P)
    for i in range(4, nsites):
        bp = (p >> (nsites - 1 - i)) & 1
        dlow += site_im[i][bp, bp]
    Loff = np.zeros((P, P))
    for i in range(4, nsites):
        bp = (p >> (nsites - 1 - i)) & 1
        Loff[p, p ^ strides[i]] += site_im[i][bp, 1 - bp]
    for T in range(NT):
        d_high = 0.0
        for i in range(4):
            bT = (T >> (3 - i)) & 1
            d_high += site_im[i][bT, bT]
            chigh[T, i] = site_im[i][bT, 1 - bT]
        Lmat = Loff.copy()
        Lmat[p, p] += d_high + dlow
        L[T] = Lmat

    Ur, Ui = U[0], U[1]
    in_maps = []
    for core in range(8):
        pg, qg = divmod(core, PC)
        tile_order = [s ^ (LT * pg) for s in range(NT)]
        rows = slice(pg * ROWS, (pg + 1) * ROWS)
        cols = slice(qg * COLS, (qg + 1) * COLS)

        # SBUF layout [p, kt, gt|ui|urn]: partition-major, packed so each
        # k-chunk loads with a single contiguous DMA
        gu_h = np.empty((P, NT, ROWS + 2 * COLS), BF)
        gt_full = (
            G[rows, :].T.reshape(NT, P, ROWS)[tile_order].transpose(1, 0, 2)
        ).astype(BF)
        gu_h[:, :, 0:512] = gt_full[:, :, 0:512]          # gtA (tl 0-3)
        gu_h[:, :, 1536:2048] = gt_full[:, :, 512:1024]   # gtB (tl 4-7)
        gu_h[:, :, 512:1024] = (
            Ui[:, cols].reshape(NT, P, COLS)[tile_order].transpose(1, 0, 2)
        ).astype(BF)
        gu_h[:, :, 1024:1536] = (
            (-Ur[:, cols]).reshape(NT, P, COLS)[tile_order].transpose(1, 0, 2)
        ).astype(BF)

        # lci[k, s, tl, m] = sign_s * L[tg][m, k]   (lhsT layout)
        tgs = [(LT * pg) ^ tl for tl in range(LT)]
        lci_h = np.empty((P, 2, LT, P), np.float64)
        ch_h = np.empty((P, 2, LT, 4), np.float32)
        for tl in range(LT):
            lci_h[:, 0, tl] = -L[tgs[tl]].T
            lci_h[:, 1, tl] = L[tgs[tl]].T
            for j in range(4):
                c = np.float32(chigh[tgs[tl], j])
                ch_h[:, 0, tl, j] = -c
                ch_h[:, 1, tl, j] = c
        in_maps.append({
            "gu": gu_h,
            "lci": lci_h.astype(BF),
            "ch": ch_h,
        })

    nc = _get_nc()
    res = run_bass_kernel_spmd(nc, in_maps, core_ids=list(range(8)), **_RUN_KWARGS)
    global _LAST_RESULT
    _LAST_RESULT = res
    out = np.empty((2, DIM, DIM), np.float32)
    for core in range(8):
        pg, qg = divmod(core, PC)
        out[:, pg * ROWS:(pg + 1) * ROWS, qg * COLS:(qg + 1) * COLS] = (
            res.results[core]["out"]
        )
    return out

